# revision 1
# baseline (speedup 1.0000x reference)
"""Causal self-attention kernel for 8 trn2 NeuronCores.

Sharding: 2 batch groups x 4 tensor-parallel ranks (Megatron-style head
split).  Core c handles batch b=c//4 and heads [4r, 4r+4) with r=c%4.
Each core:
  1. qk^T projection:   qkT[128h:(128h+128), :] = [q_h^T; k_h^T]  (64+64 rows)
  2. v projection:      v[token, 65h:65h+64], col 65h+64 = 1.0 (den trick)
  3. causal attention in s^T = [key_part, query_free] layout:
       sT = (k^T slice) matmul q^T ; p = exp(s/8) * mask ; y'T += [v|1].T p
     row 64 of y'T is the softmax denominator; normalize via reciprocal +
     partition-broadcast + multiply.
  4. partial out = y_own @ w_out[own head rows, :]  -> [2048, 1024]
  5. ReduceScatter(add) across the 4-rank group: rank r keeps the summed
     rows [512r, 512r+512); add bias -> out [512, 1024]
Host concatenates the 8 x [512, 1024] outputs into [2, 2048, 1024].
"""

import sys

for _p in ("/opt/trn_rl_repo", "/root/.axon_site", "/root/.axon_site/_ro/trn_rl_repo",
           "/root/.axon_site/_ro/pypackages"):
    if _p not in sys.path:
        sys.path.append(_p)

import numpy as np

import concourse.mybir as mybir
import concourse.tile as tile
from concourse import bacc
from concourse import bass_utils

F32 = mybir.dt.float32
BF16 = mybir.dt.bfloat16
F32R = mybir.dt.float32r


def _cfg(B=2, T=2048, C=1024, H=16, n_cores=8, tp=4):
    D = 64
    assert C == H * D
    cfg = dict(B=B, T=T, C=C, H=H, D=D, n_cores=n_cores, tp=tp)
    cfg["groups"] = [[g * tp + r for r in range(tp)] for g in range(n_cores // tp)]
    cfg["HPC"] = H // tp           # heads per core
    cfg["KT"] = C // 128           # contraction tiles for projections
    cfg["NQ"] = T // 512           # 512-wide query chunks
    cfg["TT"] = T // 128           # 128-wide token (key) tiles
    cfg["RT"] = T // tp            # output rows per core
    assert cfg["RT"] % 128 == 0 and T % 512 == 0
    return cfg


CFG = _cfg()


def build_nc(cfg=CFG, dt_mm=F32R, reps=1, no_rs=False):
    B, T, C, H, D = cfg["B"], cfg["T"], cfg["C"], cfg["H"], cfg["D"]
    HPC, KT, NQ, TT, RT = cfg["HPC"], cfg["KT"], cfg["NQ"], cfg["TT"], cfg["RT"]
    tp = cfg["tp"]
    assert HPC % 2 == 0
    Exp = mybir.ActivationFunctionType.Exp

    nc = bacc.Bacc("TRN2", target_bir_lowering=False, debug=False,
                   enable_asserts=True, num_devices=cfg["n_cores"])

    xT = nc.dram_tensor("xT", [C, T], dt_mm, kind="ExternalInput")
    w_qk = nc.dram_tensor("w_qk", [C, HPC * 128], dt_mm, kind="ExternalInput")
    w_v = nc.dram_tensor("w_v", [C, HPC * 64], dt_mm, kind="ExternalInput")
    w_out = nc.dram_tensor("w_out", [HPC * 64, C], dt_mm, kind="ExternalInput")
    b_bcast = nc.dram_tensor("b_bcast", [128, C], F32, kind="ExternalInput")
    mask = nc.dram_tensor("mask", [128, 128], dt_mm, kind="ExternalInput")
    ones = nc.dram_tensor("ones", [128, 64], dt_mm, kind="ExternalInput")
    out = nc.dram_tensor("out", [NQ * (512 // tp), C], F32, kind="ExternalOutput")

    def mm(o, lhsT, rhs, **kw):
        nc.tensor.matmul(o, lhsT, rhs, **kw)

    n_yt = (HPC * 64 + 127) // 128   # SBUF tiles holding this core's y^T
    rw = 512 // tp

    with tile.TileContext(nc) as tc:
        with (
            tc.tile_pool(name="persist", bufs=1) as per_pool,
            tc.tile_pool(name="xt", bufs=2) as xt_pool,
            tc.tile_pool(name="pT", bufs=4) as pT_pool,
            tc.tile_pool(name="norm", bufs=3) as norm_pool,
            tc.tile_pool(name="osb", bufs=4) as o_pool,
            tc.tile_pool(name="ps_s", bufs=2, space="PSUM") as ps_s,
            tc.tile_pool(name="ps_y", bufs=2, space="PSUM") as ps_y,
            tc.tile_pool(name="ps_acc", bufs=2, space="PSUM") as ps_acc,
            tc.tile_pool(name="dram", bufs=1, space="DRAM") as dram_pool,
        ):
          for _rep in range(reps):
            # emit only wqk[0] before the first x^T chunk so the first
            # matmul's inputs are at the head of the DMA queues
            wqk_sb = []
            t = per_pool.tile([128, HPC * 128], dt_mm, name="wqk0", tag="wqk0")
            nc.sync.dma_start(t[:], w_qk[0:128, :])
            wqk_sb.append(t)
            wv_sb = []
            qkT_sb = [per_pool.tile([128, 2 * T], dt_mm, name=f"qkT{hp}", tag=f"qkT{hp}")
                      for hp in range(HPC // 2)]
            v_sb = [per_pool.tile([128, HPC * 65], dt_mm, name=f"v{mt}", tag=f"v{mt}")
                    for mt in range(TT)]
            yT_sb = [per_pool.tile([128, T], dt_mm, name=f"yT{i}", tag=f"yT{i}")
                     for i in range(n_yt)]
            rs_in = [dram_pool.tile([512, C], BF16, name=f"rsi{qc}", tag=f"rsi{qc}")
                     for qc in range(NQ)]
            rs_out = [dram_pool.tile([rw, C], BF16, name=f"rso{qc}", tag=f"rso{qc}")
                      for qc in range(NQ)]

            # schedule: proj 0, proj 1, att 1, proj 2, att 2, proj 3,
            # att 3, att 0 -- the last attention chunk is the cheapest so
            # its ReduceScatter tail is minimal.
            steps = [("proj", n) for n in range(NQ)]
            steps += [("att", n) for n in
                      (list(range(1, NQ)) + [0] if NQ > 1 else [0])]
            for kind, n in steps:
              if kind == "proj":
                # ---- x^T chunk load + qk/v projections ---------------
                xt_chunk = []
                for k in range(KT):
                    t = xt_pool.tile([128, 512], dt_mm, name=f"xt{k}", tag=f"xt{k}")
                    nc.sync.dma_start(
                        t[:], xT[128 * k:128 * (k + 1), 512 * n:512 * (n + 1)])
                    xt_chunk.append(t)
                    if n == 0 and len(wqk_sb) == k + 1 and k + 1 < KT:
                        t2 = per_pool.tile([128, HPC * 128], dt_mm,
                                           name=f"wqk{k+1}", tag=f"wqk{k+1}")
                        nc.sync.dma_start(t2[:], w_qk[128 * (k+1):128 * (k + 2), :])
                        wqk_sb.append(t2)
                for m in range(HPC):
                    hp, is_k = divmod(m, 2)
                    acc = ps_acc.tile([128, 512], F32, name="acc", tag="acc")
                    for k in range(KT):
                        mm(acc[:], wqk_sb[k][:, 128 * m:128 * (m + 1)], xt_chunk[k][:],
                           start=(k == 0), stop=(k == KT - 1))
                    off = (T if is_k else 0) + 512 * n
                    # alternate eviction engines so psum slots free faster
                    if m % 2 == 0:
                        nc.scalar.copy(qkT_sb[hp][:, off:off + 512], acc[:])
                    else:
                        nc.vector.tensor_copy(qkT_sb[hp][:, off:off + 512], acc[:])
                if n == 0:
                    for k in range(KT):
                        t = per_pool.tile([128, HPC * 64], dt_mm, name=f"wv{k}",
                                          tag=f"wv{k}")
                        nc.sync.dma_start(t[:], w_v[128 * k:128 * (k + 1), :])
                        wv_sb.append(t)
                    ones_sb = per_pool.tile([128, 64], dt_mm, name="ones", tag="ones")
                    nc.sync.dma_start(ones_sb[:], ones[:, :])
                for j in range(4):
                    mt = 4 * n + j
                    acc = ps_acc.tile([128, HPC * 64], F32, name="acc", tag="acc")
                    for k in range(KT):
                        mm(acc[:], xt_chunk[k][:, 128 * j:128 * (j + 1)], wv_sb[k][:],
                           start=(k == 0), stop=(k == KT - 1))
                    vt = v_sb[mt]
                    vsrc = acc[:].rearrange("p (h e) -> p h e", e=64)
                    vdst = vt[:].rearrange("p (h e) -> p h e", e=65)[:, :, 0:64]
                    nc.vector.tensor_copy(vdst, vsrc)
                    nc.vector.tensor_copy(
                        vt[:].rearrange("p (h e) -> p h e", e=65)[:, :, 64:65],
                        ones_sb[:, 0:HPC].rearrange("p (h e) -> p h e", e=1))
                if n == 0:
                    msk_sb = per_pool.tile([128, 128], dt_mm, name="mask", tag="mask")
                    nc.sync.dma_start(msk_sb[:], mask[:, :])
                    bb_sb = per_pool.tile([128, C], F32, name="bb", tag="bb")
                    nc.sync.dma_start(bb_sb[:], b_bcast[:, :])
                    wout_sb = []
                    for k in range(n_yt):
                        rows = min(128, HPC * 64 - 128 * k)
                        t = per_pool.tile([rows, C], dt_mm, name=f"wout{k}",
                                          tag=f"wout{k}")
                        nc.sync.dma_start(t[:], w_out[128 * k:128 * k + rows, :])
                        wout_sb.append(t)

                continue
              else:
                # ---- attention for query chunk qc = n ----------------
                qc = n
                for h in range(HPC):
                    hp, half = divmod(h, 2)
                    base = 64 * half
                    qT = qkT_sb[hp][base:base + 64, 0:T]
                    kT = qkT_sb[hp][base:base + 64, T:2 * T]
                    y_acc = ps_y.tile([65, 512], F32, name="y", tag="y")
                    n_kt = 4 * qc + 4
                    # non-diagonal tiles in pairs (one exp per pair)
                    kt = 0
                    first = True
                    while kt < 4 * qc:
                        s_ps = ps_s.tile([128, 1024], F32, name="s", tag="s")
                        pT = pT_pool.tile([128, 1024], dt_mm, name="p", tag="p")
                        for half_i in range(2):
                            mm(s_ps[:, 512 * half_i:512 * (half_i + 1)],
                               kT[:, 128 * (kt + half_i):128 * (kt + half_i + 1)],
                               qT[:, 512 * qc:512 * (qc + 1)],
                               start=True, stop=True)
                        nc.scalar.activation(pT[:], s_ps[:], Exp, scale=0.125)
                        for half_i in range(2):
                            mm(y_acc[:], v_sb[kt + half_i][:, 65 * h:65 * h + 65],
                               pT[:, 512 * half_i:512 * (half_i + 1)],
                               start=first, stop=False)
                            first = False
                        kt += 2
                    # diagonal tiles: restrict to valid columns
                    for i in range(4):
                        ktd = 4 * qc + i
                        lo = 128 * i
                        s_ps = ps_s.tile([128, 1024], F32, name="s", tag="s")
                        pT = pT_pool.tile([128, 1024], dt_mm, name="p", tag="p")
                        mm(s_ps[:, lo:512], kT[:, 128 * ktd:128 * (ktd + 1)],
                           qT[:, 512 * qc + lo:512 * (qc + 1)],
                           start=True, stop=True)
                        nc.scalar.activation(pT[:, lo:512], s_ps[:, lo:512],
                                             Exp, scale=0.125)
                        nc.vector.tensor_mul(
                            pT[:, lo:lo + 128], pT[:, lo:lo + 128], msk_sb[:])
                        mm(y_acc[:, lo:512], v_sb[ktd][:, 65 * h:65 * h + 65],
                           pT[:, lo:512],
                           start=first, stop=(i == 3))
                        first = False
                    # normalize: row 64 of y_acc is the denominator
                    r_sb = norm_pool.tile([1, 512], F32, name="r", tag="r")
                    nc.vector.reciprocal(r_sb[:], y_acc[64:65, :])
                    r_dram = dram_pool.tile([1, 512], F32, name="rd", tag="rd", bufs=2)
                    nc.sync.dma_start(r_dram[:], r_sb[:])
                    rb_sb = norm_pool.tile([64, 512], F32, name="rb", tag="rb")
                    nc.sync.dma_start(rb_sb[:], r_dram[:].to_broadcast((64, 512)))
                    ti, po = divmod(64 * h, 128)
                    nc.vector.tensor_mul(
                        yT_sb[ti][po:po + 64, 512 * qc:512 * (qc + 1)],
                        y_acc[0:64, :], rb_sb[:])

                # ---- out-proj for this chunk + ReduceScatter ---------
                for j in range(4):
                    m = 4 * qc + j
                    for nn in range(C // 512):
                        acc = ps_acc.tile([128, 512], F32, name="acc", tag="acc")
                        for k in range(n_yt):
                            mm(acc[:], yT_sb[k][:, 128 * m:128 * (m + 1)],
                               wout_sb[k][:, 512 * nn:512 * (nn + 1)],
                               start=(k == 0), stop=(k == n_yt - 1))
                        po_sb = o_pool.tile([128, 512], BF16, name="po", tag="po")
                        nc.vector.tensor_add(po_sb[:], acc[:],
                                             bb_sb[:, 512 * nn:512 * (nn + 1)])
                        nc.sync.dma_start(
                            rs_in[qc][128 * j:128 * (j + 1), 512 * nn:512 * (nn + 1)],
                            po_sb[:])
                if no_rs:
                    nc.sync.dma_start(rs_out[qc][:], rs_in[qc][0:rw, :])
                else:
                    nc.gpsimd.collective_compute(
                        "ReduceScatter", mybir.AluOpType.add,
                        replica_groups=cfg["groups"],
                        ins=[rs_in[qc][:].opt()], outs=[rs_out[qc][:].opt()])
                # bf16 -> f32 via a single casting SWDGE DMA
                nc.gpsimd.dma_start(
                    out[rw * qc:rw * (qc + 1), :].rearrange("p f -> () (p f)"),
                    rs_out[qc][:].rearrange("p f -> () (p f)"))
    nc.compile()
    return nc


def shard_inputs(x, w_qkv, w_out, b_out, cfg=CFG):
    B, T, C, H, D, tp = (cfg["B"], cfg["T"], cfg["C"], cfg["H"], cfg["D"], cfg["tp"])
    HPC = cfg["HPC"]
    x = np.asarray(x, dtype=np.float32)
    w_qkv = np.asarray(w_qkv, dtype=np.float32)
    w_out = np.asarray(w_out, dtype=np.float32)
    b_out = np.asarray(b_out, dtype=np.float32)

    w_q, w_k, w_v = w_qkv[:, :C], w_qkv[:, C:2 * C], w_qkv[:, 2 * C:]
    kp = np.arange(128)[:, None]
    qf = np.arange(128)[None, :]
    mask = (kp <= qf).astype(np.float32)
    b_bcast = np.ascontiguousarray(np.broadcast_to(b_out / tp, (128, C)))

    in_maps = []
    for c in range(cfg["n_cores"]):
        b, r = divmod(c, tp)
        heads = range(HPC * r, HPC * (r + 1))
        heads = list(heads)
        blocks = []
        for hp in range(len(heads) // 2):
            g0, g1 = heads[2 * hp], heads[2 * hp + 1]
            blocks.append(np.concatenate(
                [w_q[:, 64 * g0:64 * (g0 + 1)], w_q[:, 64 * g1:64 * (g1 + 1)]], axis=1))
            blocks.append(np.concatenate(
                [w_k[:, 64 * g0:64 * (g0 + 1)], w_k[:, 64 * g1:64 * (g1 + 1)]], axis=1))
        wqk_c = np.concatenate(blocks, axis=1)
        wv_c = np.concatenate([w_v[:, 64 * g:64 * (g + 1)] for g in heads], axis=1)
        wout_c = np.concatenate([w_out[64 * g:64 * (g + 1), :] for g in heads], axis=0)
        in_maps.append({
            "xT": np.ascontiguousarray(x[b].T),
            "w_qk": np.ascontiguousarray(wqk_c),
            "w_v": np.ascontiguousarray(wv_c),
            "w_out": np.ascontiguousarray(wout_c),
            "b_bcast": b_bcast,
            "mask": mask,
            "ones": np.ones((128, 64), dtype=np.float32),
        })
    return in_maps


def assemble(results, cfg=CFG):
    B, T, C, tp, NQ = cfg["B"], cfg["T"], cfg["C"], cfg["tp"], cfg["NQ"]
    rw = 512 // tp
    out = np.empty((B, T, C), dtype=np.float32)
    for c in range(cfg["n_cores"]):
        b, r = divmod(c, tp)
        o = results[c]["out"]
        for qc in range(NQ):
            out[b, 512 * qc + rw * r:512 * qc + rw * (r + 1), :] = \
                o[rw * qc:rw * (qc + 1)]
    return out


_NC_CACHE = {}


def _get_nc(cfg_key="default", cfg=CFG):
    if cfg_key not in _NC_CACHE:
        _NC_CACHE[cfg_key] = build_nc(cfg)
    return _NC_CACHE[cfg_key]


def kernel(x, w_qkv, w_out, b_out):
    cfg = CFG
    nc = _get_nc()
    in_maps = shard_inputs(x, w_qkv, w_out, b_out, cfg)
    res = bass_utils.run_bass_kernel_spmd(
        nc, in_maps, core_ids=list(range(cfg["n_cores"])))
    return assemble(res.results, cfg)


if __name__ == "__main__":
    print("module loads ok")



# revision 4
# speedup vs baseline: 1.6837x; 1.6837x over previous
"""Causal self-attention kernel for 8 trn2 NeuronCores.

Sharding: 2 batch groups x 4 tensor-parallel ranks (Megatron-style head
split).  Core c handles batch b=c//4 and heads [4r, 4r+4) with r=c%4.
Each core:
  1. qk^T projection:   qkT[128h:(128h+128), :] = [q_h^T; k_h^T]  (64+64 rows)
  2. v projection:      v[token, 65h:65h+64], col 65h+64 = 1.0 (den trick)
  3. causal attention in s^T = [key_part, query_free] layout:
       sT = (k^T slice) matmul q^T ; p = exp(s/8) * mask ; y'T += [v|1].T p
     row 64 of y'T is the softmax denominator; normalize via reciprocal +
     partition-broadcast + multiply.
  4. partial out = y_own @ w_out[own head rows, :]  -> [2048, 1024]
  5. ReduceScatter(add) across the 4-rank group: rank r keeps the summed
     rows [512r, 512r+512); add bias -> out [512, 1024]
Host concatenates the 8 x [512, 1024] outputs into [2, 2048, 1024].
"""

import sys

for _p in ("/opt/trn_rl_repo", "/root/.axon_site", "/root/.axon_site/_ro/trn_rl_repo",
           "/root/.axon_site/_ro/pypackages"):
    if _p not in sys.path:
        sys.path.append(_p)

import numpy as np

import concourse.mybir as mybir
import concourse.tile as tile
from concourse import bacc
from concourse import bass_utils

F32 = mybir.dt.float32
BF16 = mybir.dt.bfloat16
F32R = mybir.dt.float32r


def _cfg(B=2, T=2048, C=1024, H=16, n_cores=8, tp=4):
    D = 64
    assert C == H * D
    cfg = dict(B=B, T=T, C=C, H=H, D=D, n_cores=n_cores, tp=tp)
    cfg["groups"] = [[g * tp + r for r in range(tp)] for g in range(n_cores // tp)]
    cfg["HPC"] = H // tp           # heads per core
    cfg["KT"] = C // 128           # contraction tiles for projections
    cfg["NQ"] = T // 512           # 512-wide query chunks
    cfg["TT"] = T // 128           # 128-wide token (key) tiles
    cfg["RT"] = T // tp            # output rows per core
    assert cfg["RT"] % 128 == 0 and T % 512 == 0
    return cfg


CFG = _cfg()


def build_nc(cfg=CFG, dt_mm=BF16, reps=1, no_rs=False):
    B, T, C, H, D = cfg["B"], cfg["T"], cfg["C"], cfg["H"], cfg["D"]
    HPC, KT, NQ, TT, RT = cfg["HPC"], cfg["KT"], cfg["NQ"], cfg["TT"], cfg["RT"]
    tp = cfg["tp"]
    assert HPC % 2 == 0
    Exp = mybir.ActivationFunctionType.Exp

    nc = bacc.Bacc("TRN2", target_bir_lowering=False, debug=False,
                   enable_asserts=True, num_devices=cfg["n_cores"])

    xT = nc.dram_tensor("xT", [C, T], dt_mm, kind="ExternalInput")
    w_qk = nc.dram_tensor("w_qk", [C, HPC * 128], dt_mm, kind="ExternalInput")
    w_v = nc.dram_tensor("w_v", [C, HPC * 64], dt_mm, kind="ExternalInput")
    w_out = nc.dram_tensor("w_out", [HPC * 64, C], dt_mm, kind="ExternalInput")
    b_bcast = nc.dram_tensor("b_bcast", [128, C], F32, kind="ExternalInput")
    mask = nc.dram_tensor("mask", [128, 128], dt_mm, kind="ExternalInput")
    ones = nc.dram_tensor("ones", [128, 64], dt_mm, kind="ExternalInput")
    out = nc.dram_tensor("out", [NQ * (512 // tp), C], F32, kind="ExternalOutput")

    def mm(o, lhsT, rhs, **kw):
        nc.tensor.matmul(o, lhsT, rhs, **kw)

    n_yt = (HPC * 64 + 127) // 128   # SBUF tiles holding this core's y^T
    rw = 512 // tp

    with tile.TileContext(nc) as tc:
        with (
            tc.tile_pool(name="persist", bufs=1) as per_pool,
            tc.tile_pool(name="xt", bufs=2) as xt_pool,
            tc.tile_pool(name="pT", bufs=4) as pT_pool,
            tc.tile_pool(name="norm", bufs=3) as norm_pool,
            tc.tile_pool(name="osb", bufs=4) as o_pool,
            tc.tile_pool(name="ps_s", bufs=2, space="PSUM") as ps_s,
            tc.tile_pool(name="ps_y", bufs=2, space="PSUM") as ps_y,
            tc.tile_pool(name="ps_acc", bufs=2, space="PSUM") as ps_acc,
            tc.tile_pool(name="dram", bufs=1, space="DRAM") as dram_pool,
        ):
          for _rep in range(reps):
            # emit only wqk[0] before the first x^T chunk so the first
            # matmul's inputs are at the head of the DMA queues
            wqk_sb = []
            t = per_pool.tile([128, HPC * 128], dt_mm, name="wqk0", tag="wqk0")
            nc.sync.dma_start(t[:], w_qk[0:128, :])
            wqk_sb.append(t)
            wv_sb = []
            qkT_sb = [per_pool.tile([128, 2 * T], dt_mm, name=f"qkT{hp}", tag=f"qkT{hp}")
                      for hp in range(HPC // 2)]
            v_sb = [per_pool.tile([128, HPC * 65], dt_mm, name=f"v{mt}", tag=f"v{mt}")
                    for mt in range(TT)]
            yT_sb = [per_pool.tile([128, T], dt_mm, name=f"yT{i}", tag=f"yT{i}")
                     for i in range(n_yt)]
            rs_in = [dram_pool.tile([512, C], BF16, name=f"rsi{qc}", tag=f"rsi{qc}")
                     for qc in range(NQ)]
            rs_out = [dram_pool.tile([rw, C], BF16, name=f"rso{qc}", tag=f"rso{qc}")
                      for qc in range(NQ)]

            # schedule: proj 0, proj 1, att 1, proj 2, att 2, proj 3,
            # att 3, att 0 -- the last attention chunk is the cheapest so
            # its ReduceScatter tail is minimal.
            steps = [("proj", n) for n in range(NQ)]
            steps += [("att", n) for n in
                      (list(range(1, NQ)) + [0] if NQ > 1 else [0])]
            for kind, n in steps:
              if kind == "proj":
                # ---- x^T chunk load + qk/v projections ---------------
                xt_chunk = []
                for k in range(KT):
                    t = xt_pool.tile([128, 512], dt_mm, name=f"xt{k}", tag=f"xt{k}")
                    nc.sync.dma_start(
                        t[:], xT[128 * k:128 * (k + 1), 512 * n:512 * (n + 1)])
                    xt_chunk.append(t)
                    if n == 0 and len(wqk_sb) == k + 1 and k + 1 < KT:
                        t2 = per_pool.tile([128, HPC * 128], dt_mm,
                                           name=f"wqk{k+1}", tag=f"wqk{k+1}")
                        nc.sync.dma_start(t2[:], w_qk[128 * (k+1):128 * (k + 2), :])
                        wqk_sb.append(t2)
                for m in range(HPC):
                    hp, is_k = divmod(m, 2)
                    acc = ps_acc.tile([128, 512], F32, name="acc", tag="acc")
                    for k in range(KT):
                        mm(acc[:], wqk_sb[k][:, 128 * m:128 * (m + 1)], xt_chunk[k][:],
                           start=(k == 0), stop=(k == KT - 1))
                    off = (T if is_k else 0) + 512 * n
                    # alternate eviction engines so psum slots free faster
                    if m % 2 == 0:
                        nc.scalar.copy(qkT_sb[hp][:, off:off + 512], acc[:])
                    else:
                        nc.vector.tensor_copy(qkT_sb[hp][:, off:off + 512], acc[:])
                if n == 0:
                    for k in range(KT):
                        t = per_pool.tile([128, HPC * 64], dt_mm, name=f"wv{k}",
                                          tag=f"wv{k}")
                        nc.sync.dma_start(t[:], w_v[128 * k:128 * (k + 1), :])
                        wv_sb.append(t)
                    ones_sb = per_pool.tile([128, 64], dt_mm, name="ones", tag="ones")
                    nc.sync.dma_start(ones_sb[:], ones[:, :])
                for j in range(4):
                    mt = 4 * n + j
                    acc = ps_acc.tile([128, HPC * 64], F32, name="acc", tag="acc")
                    for k in range(KT):
                        mm(acc[:], xt_chunk[k][:, 128 * j:128 * (j + 1)], wv_sb[k][:],
                           start=(k == 0), stop=(k == KT - 1))
                    vt = v_sb[mt]
                    vsrc = acc[:].rearrange("p (h e) -> p h e", e=64)
                    vdst = vt[:].rearrange("p (h e) -> p h e", e=65)[:, :, 0:64]
                    nc.vector.tensor_copy(vdst, vsrc)
                    nc.vector.tensor_copy(
                        vt[:].rearrange("p (h e) -> p h e", e=65)[:, :, 64:65],
                        ones_sb[:, 0:HPC].rearrange("p (h e) -> p h e", e=1))
                if n == 0:
                    msk_sb = per_pool.tile([128, 128], dt_mm, name="mask", tag="mask")
                    nc.sync.dma_start(msk_sb[:], mask[:, :])
                    bb_sb = per_pool.tile([128, C], F32, name="bb", tag="bb")
                    nc.sync.dma_start(bb_sb[:], b_bcast[:, :])
                    wout_sb = []
                    for k in range(n_yt):
                        rows = min(128, HPC * 64 - 128 * k)
                        t = per_pool.tile([rows, C], dt_mm, name=f"wout{k}",
                                          tag=f"wout{k}")
                        nc.sync.dma_start(t[:], w_out[128 * k:128 * k + rows, :])
                        wout_sb.append(t)

                continue
              else:
                # ---- attention for query chunk qc = n ----------------
                qc = n
                for h in range(HPC):
                    hp, half = divmod(h, 2)
                    base = 64 * half
                    qT = qkT_sb[hp][base:base + 64, 0:T]
                    kT = qkT_sb[hp][base:base + 64, T:2 * T]
                    y_acc = ps_y.tile([65, 512], F32, name="y", tag="y")
                    n_kt = 4 * qc + 4
                    # non-diagonal tiles in pairs (one exp per pair)
                    kt = 0
                    first = True
                    while kt < 4 * qc:
                        s_ps = ps_s.tile([128, 1024], F32, name="s", tag="s")
                        pT = pT_pool.tile([128, 1024], dt_mm, name="p", tag="p")
                        for half_i in range(2):
                            mm(s_ps[:, 512 * half_i:512 * (half_i + 1)],
                               kT[:, 128 * (kt + half_i):128 * (kt + half_i + 1)],
                               qT[:, 512 * qc:512 * (qc + 1)],
                               start=True, stop=True)
                        nc.scalar.activation(pT[:], s_ps[:], Exp, scale=0.125)
                        for half_i in range(2):
                            mm(y_acc[:], v_sb[kt + half_i][:, 65 * h:65 * h + 65],
                               pT[:, 512 * half_i:512 * (half_i + 1)],
                               start=first, stop=False)
                            first = False
                        kt += 2
                    # diagonal tiles: restrict to valid columns
                    for i in range(4):
                        ktd = 4 * qc + i
                        lo = 128 * i
                        s_ps = ps_s.tile([128, 1024], F32, name="s", tag="s")
                        pT = pT_pool.tile([128, 1024], dt_mm, name="p", tag="p")
                        mm(s_ps[:, lo:512], kT[:, 128 * ktd:128 * (ktd + 1)],
                           qT[:, 512 * qc + lo:512 * (qc + 1)],
                           start=True, stop=True)
                        nc.scalar.activation(pT[:, lo:512], s_ps[:, lo:512],
                                             Exp, scale=0.125)
                        nc.vector.tensor_mul(
                            pT[:, lo:lo + 128], pT[:, lo:lo + 128], msk_sb[:])
                        mm(y_acc[:, lo:512], v_sb[ktd][:, 65 * h:65 * h + 65],
                           pT[:, lo:512],
                           start=first, stop=(i == 3))
                        first = False
                    # normalize: row 64 of y_acc is the denominator
                    r_sb = norm_pool.tile([1, 512], F32, name="r", tag="r")
                    nc.vector.reciprocal(r_sb[:], y_acc[64:65, :])
                    r_dram = dram_pool.tile([1, 512], F32, name="rd", tag="rd", bufs=2)
                    nc.sync.dma_start(r_dram[:], r_sb[:])
                    rb_sb = norm_pool.tile([64, 512], F32, name="rb", tag="rb")
                    nc.sync.dma_start(rb_sb[:], r_dram[:].to_broadcast((64, 512)))
                    ti, po = divmod(64 * h, 128)
                    nc.vector.tensor_mul(
                        yT_sb[ti][po:po + 64, 512 * qc:512 * (qc + 1)],
                        y_acc[0:64, :], rb_sb[:])

                # ---- out-proj for this chunk + ReduceScatter ---------
                for j in range(4):
                    m = 4 * qc + j
                    for nn in range(C // 512):
                        acc = ps_acc.tile([128, 512], F32, name="acc", tag="acc")
                        for k in range(n_yt):
                            mm(acc[:], yT_sb[k][:, 128 * m:128 * (m + 1)],
                               wout_sb[k][:, 512 * nn:512 * (nn + 1)],
                               start=(k == 0), stop=(k == n_yt - 1))
                        po_sb = o_pool.tile([128, 512], BF16, name="po", tag="po")
                        nc.vector.tensor_add(po_sb[:], acc[:],
                                             bb_sb[:, 512 * nn:512 * (nn + 1)])
                        nc.sync.dma_start(
                            rs_in[qc][128 * j:128 * (j + 1), 512 * nn:512 * (nn + 1)],
                            po_sb[:])
                if no_rs:
                    nc.sync.dma_start(rs_out[qc][:], rs_in[qc][0:rw, :])
                else:
                    nc.gpsimd.collective_compute(
                        "ReduceScatter", mybir.AluOpType.add,
                        replica_groups=cfg["groups"],
                        ins=[rs_in[qc][:].opt()], outs=[rs_out[qc][:].opt()])
                # bf16 -> f32 via a single casting SWDGE DMA
                nc.gpsimd.dma_start(
                    out[rw * qc:rw * (qc + 1), :].rearrange("p f -> () (p f)"),
                    rs_out[qc][:].rearrange("p f -> () (p f)"))
    nc.compile()
    return nc


def shard_inputs(x, w_qkv, w_out, b_out, cfg=CFG):
    import ml_dtypes
    bf16 = ml_dtypes.bfloat16
    B, T, C, H, D, tp = (cfg["B"], cfg["T"], cfg["C"], cfg["H"], cfg["D"], cfg["tp"])
    HPC = cfg["HPC"]
    x = np.asarray(x, dtype=np.float32).astype(bf16)
    w_qkv = np.asarray(w_qkv, dtype=np.float32).astype(bf16)
    w_out = np.asarray(w_out, dtype=np.float32).astype(bf16)
    b_out = np.asarray(b_out, dtype=np.float32)

    w_q, w_k, w_v = w_qkv[:, :C], w_qkv[:, C:2 * C], w_qkv[:, 2 * C:]
    kp = np.arange(128)[:, None]
    qf = np.arange(128)[None, :]
    mask = (kp <= qf).astype(bf16)
    b_bcast = np.ascontiguousarray(np.broadcast_to(b_out / tp, (128, C)))

    in_maps = []
    for c in range(cfg["n_cores"]):
        b, r = divmod(c, tp)
        heads = range(HPC * r, HPC * (r + 1))
        heads = list(heads)
        blocks = []
        for hp in range(len(heads) // 2):
            g0, g1 = heads[2 * hp], heads[2 * hp + 1]
            blocks.append(np.concatenate(
                [w_q[:, 64 * g0:64 * (g0 + 1)], w_q[:, 64 * g1:64 * (g1 + 1)]], axis=1))
            blocks.append(np.concatenate(
                [w_k[:, 64 * g0:64 * (g0 + 1)], w_k[:, 64 * g1:64 * (g1 + 1)]], axis=1))
        wqk_c = np.concatenate(blocks, axis=1)
        wv_c = np.concatenate([w_v[:, 64 * g:64 * (g + 1)] for g in heads], axis=1)
        wout_c = np.concatenate([w_out[64 * g:64 * (g + 1), :] for g in heads], axis=0)
        in_maps.append({
            "xT": np.ascontiguousarray(x[b].T),
            "w_qk": np.ascontiguousarray(wqk_c),
            "w_v": np.ascontiguousarray(wv_c),
            "w_out": np.ascontiguousarray(wout_c),
            "b_bcast": b_bcast,
            "mask": mask,
            "ones": np.ones((128, 64), dtype=bf16),
        })
    return in_maps


def assemble(results, cfg=CFG):
    B, T, C, tp, NQ = cfg["B"], cfg["T"], cfg["C"], cfg["tp"], cfg["NQ"]
    rw = 512 // tp
    out = np.empty((B, T, C), dtype=np.float32)
    for c in range(cfg["n_cores"]):
        b, r = divmod(c, tp)
        o = results[c]["out"]
        for qc in range(NQ):
            out[b, 512 * qc + rw * r:512 * qc + rw * (r + 1), :] = \
                o[rw * qc:rw * (qc + 1)]
    return out


_NC_CACHE = {}


def _get_nc(cfg_key="default", cfg=CFG):
    if cfg_key not in _NC_CACHE:
        _NC_CACHE[cfg_key] = build_nc(cfg)
    return _NC_CACHE[cfg_key]


def kernel(x, w_qkv, w_out, b_out):
    cfg = CFG
    nc = _get_nc()
    in_maps = shard_inputs(x, w_qkv, w_out, b_out, cfg)
    res = bass_utils.run_bass_kernel_spmd(
        nc, in_maps, core_ids=list(range(cfg["n_cores"])))
    return assemble(res.results, cfg)


if __name__ == "__main__":
    print("module loads ok")

